# revision 61
# baseline (speedup 1.0000x reference)
"""GNN message-passing kernel for Trainium2 (8 NeuronCores).

Reference computation:
    out[b,i,f] = X[b,0,i,i,f] + sum_{k=1..3} sum_j A[b,i,j] * X[b,k,i,j,f]

Sharding: 8 cores = (batch b in 0..3) x (i-half h in 0..1); each core owns
a (b, 128-row i-slab) of the output. Hop 0 only contributes its diagonal,
so only X[b,1:4] (3/4 of X) plus the hop-0 diagonal rows are ever sent to
the device. X is converted to bf16 on the host (rel tol is 2e-2; the
full pipeline measures ~7e-3), halving DMA traffic to ~12.6 MB per core,
and re-laid-out chunk-major so each j-chunk is one fully contiguous DMA.
~12.6 MB / ~360 GB/s HBM-per-core is the ~35 us floor this kernel tracks.

Per-core device pipeline (chunks of CJ j-columns; DMA -> PE -> ACT -> DVE):
  - Every chunk has its OWN SBUF buffer (the whole bf16 X slab fits:
    96 KB/partition), so no DMA trigger ever waits on a buffer and the
    stream runs at pure HBM rate regardless of compute hiccups -- this
    decouples the engines and kills backpressure-induced variance.
  - TensorE: identity-stationary bf16 matmuls (single pass) accumulate
    x1+x2+x3 into PSUM fp32. A garbage-weight warm-up burst (memset tile,
    no DMA dependency) trips the HAM clock gate to 2.4 GHz early.
  - ScalarE (otherwise idle): copies the PSUM fp32 hop sum to SBUF bf16.
  - DVE: expands A[i,j] -> expA[i, j*F+f] during the fill (int32
    pair-packed copy, quartered so nothing blocks), then per chunk ONE
    unit-stride bf16 multiply (2x mode; no broadcast AP / PSUM operand,
    either of which would force 1x) and ONE bf16 add into a running
    1024-wide accumulator (+ a fold for 2048-wide chunks); the
    j-reduction finishes in a short final fold, pre-folded while the
    tail chunks stream.

Measured on 8 axon-tunneled trn2 cores: ~54-56 us HW exec typical
(53.9 us best; run-to-run HBM contention moves the stream between
~32-40 us), rel err 6.4e-3. Baseline this replaced: 112 us (fp32,
DVE-bound). Rejected on measurement: 1024-col matmuls (ISA cap 512),
ldweights=False dedup (walrus ignores it), 64-col or all-zero warm-up
matmuls (don't trip HAM -- too few subarrays / zero-skipped), paired
64j DMA transfers (slower in practice), per-matmul LDWEIGHTS is free
once HAM is warm (reorder window hides it).
"""

import sys

if "/opt/trn_rl_repo" not in sys.path:
    sys.path.insert(0, "/opt/trn_rl_repo")

import ml_dtypes
import numpy as np

import concourse.bacc as bacc
import concourse.bass as bass
import concourse.mybir as mybir
from concourse.bass_utils import run_bass_kernel_spmd
from concourse.tile import TileContext

BATCH, KP1, N, F = 4, 4, 256, 64
NH = N // 2          # 128 rows of output per core (partition dim)
CJS = [8, 8, 16] + [32] * 6 + [16, 8, 8]  # sum = 256; small head + tail
assert sum(CJS) == N
MMCOL = 512          # moving columns per matmul (ISA max)
ACCW = 1024          # running accumulator width (elements per partition)
FP32 = mybir.dt.float32
BF16 = mybir.dt.bfloat16
INT32 = mybir.dt.int32
BF16_NP = ml_dtypes.bfloat16

_CACHE = {}


def _build_nc():
    if "nc" in _CACHE:
        return _CACHE["nc"]
    nc = bacc.Bacc("TRN2", target_bir_lowering=False, debug=False, num_devices=8)
    # chunk-major: all of chunk c (128 i x 3 hops x CJ j x F) contiguous
    xk = nc.dram_tensor("xk", [NH * 3 * N * F], BF16, kind="ExternalInput").ap()
    a2 = nc.dram_tensor("a2", [NH, N], INT32, kind="ExternalInput").ap()
    d = nc.dram_tensor("d", [NH, F], FP32, kind="ExternalInput").ap()
    eye = nc.dram_tensor("eye", [128, 128], BF16, kind="ExternalInput").ap()
    out = nc.dram_tensor("out", [NH, F], FP32, kind="ExternalOutput").ap()

    FH = F // 2  # int32 pairs per j in the expanded-A row
    n_small = sum(1 for cj in CJS if cj < 32)
    n_big = sum(1 for cj in CJS if cj >= 32)

    with TileContext(nc) as tc:
        with (
            tc.tile_pool(name="const", bufs=1) as cpool,
            tc.tile_pool(name="xss", bufs=n_small) as xspool,
            tc.tile_pool(name="xsb", bufs=n_big) as xbpool,
            tc.tile_pool(name="sm", bufs=2) as smpool,
            tc.tile_pool(name="pr", bufs=2) as prpool,
            tc.tile_pool(name="ac", bufs=1) as acpool,
            tc.tile_pool(name="ps", bufs=2, space="PSUM") as pspool,
        ):
            # eye/a2 go FIRST on the sync ring so their packets complete
            # before the big chunk transfers occupy the shared SDMA engines.
            eye_sb = cpool.tile([128, 128], BF16)
            nc.sync.dma_start(out=eye_sb[:, :], in_=eye[:, :])
            a2_sb = cpool.tile([128, N], INT32)
            nc.sync.dma_start(out=a2_sb[:, :], in_=a2[:, :])
            d_sb = cpool.tile([128, F], FP32)
            # d is only needed at the end: ACT ring, off the critical path
            nc.scalar.dma_start(out=d_sb[:, :], in_=d[:, :])

            expa = cpool.tile([128, N * FH], INT32)  # = [128, N*F] bf16
            acc = acpool.tile([128, ACCW], BF16)
            nc.vector.memset(acc[:, :], 0.0)
            accf = acpool.tile([128, F], FP32)

            # A-expansion (DVE): expa[i, j*FH + q] = a2[i, j] (int32 =
            # packed bf16 pair), quartered so the first multiply is not
            # blocked behind one long copy.
            e_step = expa.ap[0][0]
            a_step = a2_sb.ap[0][0]

            def expand_quarter(q):
                NQ = N // 4
                eo = bass.AP(
                    expa.tensor, q * NQ * FH, [[e_step, 128], [FH, NQ], [1, FH]]
                )
                ei = bass.AP(
                    a2_sb.tensor, q * NQ, [[a_step, 128], [1, NQ], [0, FH]]
                )
                nc.vector.tensor_copy(eo, ei)

            expand_quarter(0)

            # PE warm-up on a memset tile: no DMA dependency, so the burst
            # starts at engine boot and HAM reaches 2.4 GHz early. Full
            # 128x128 matmuls -- narrow ones don't trip the detector.
            garbage = cpool.tile([128, 128], BF16)
            nc.gpsimd.memset(garbage[:, :], 0.5)
            warm = pspool.tile([128, max(CJS) * F], FP32, name="ps", tag="ps")
            for _ in range(24):
                nc.tensor.matmul(
                    warm[:, 0:128],
                    garbage[:, :],
                    garbage[:, :],
                    start=True,
                    stop=True,
                )

            # DMA granularity is decoupled from compute granularity: the six
            # 32j body chunks arrive as three 64j transfers (fewer DMA
            # fixed-overhead bubbles in the stream); compute still runs in
            # 32j chunks (PSUM-sized), reading halves of the shared tile.
            xoff = 0
            for c, CJ in enumerate(CJS):
                CF = CJ * F
                pool = xbpool if CJ >= 32 else xspool
                xt = pool.tile([128, 3 * CF], BF16, name="xt", tag="xt")
                src = bass.AP(xk.tensor, xoff, [[3 * CF, 128], [1, 3 * CF]])
                nc.sync.dma_start(out=xt[:, :], in_=src)
                xoff += 128 * 3 * CF
                hs, xbase = CF, 0

                # hop sum on TensorEngine: bf16 identity matmuls, PSUM fp32
                ps = pspool.tile([128, CF], FP32, name="ps", tag="ps")
                wmm = min(MMCOL, CF)
                for s in range(CF // wmm):
                    sl = slice(s * wmm, (s + 1) * wmm)
                    for k in range(3):
                        nc.tensor.matmul(
                            ps[:, sl],
                            eye_sb[:, :],
                            xt[
                                :,
                                xbase + k * hs + s * wmm : xbase
                                + k * hs
                                + (s + 1) * wmm,
                            ],
                            start=(k == 0),
                            stop=(k == 2),
                        )

                # DVE multiply by expA. Mid-stream chunks go through a
                # ScalarE PSUM->SBUF bf16 copy first so the mul runs in 2x
                # mode; the LAST (tiny) chunk skips that stage -- a 1x
                # PSUM-source mul is cheaper than an extra cross-engine
                # latency hop on the drain-critical path.
                j0 = sum(CJS[:c])
                prod = prpool.tile([128, CF], BF16, name="prod", tag="prod")
                ea = expa[:, j0 * FH : (j0 + CJ) * FH].bitcast(BF16)
                if c == len(CJS) - 1:
                    nc.vector.tensor_mul(prod[:, :], ps[:, :], ea)
                else:
                    s_sb = smpool.tile([128, CF], BF16, name="ssb", tag="ssb")
                    nc.scalar.copy(s_sb[:, :], ps[:, :])
                    nc.vector.tensor_mul(prod[:, :], s_sb[:, :], ea)
                if c == 5:
                    # hop-0 diagonal folded in mid-stream (DVE has slack
                    # here) instead of on the drain-critical final chain
                    nc.vector.tensor_add(acc[:, 0:F], acc[:, 0:F], d_sb[:, :])

                # fold prod down to the acc width, then accumulate (a
                # narrower prod adds into a prefix -- column sums survive)
                w = CF
                while w > ACCW:
                    h = w // 2
                    nc.vector.tensor_add(prod[:, 0:h], prod[:, 0:h], prod[:, h:w])
                    w = h
                nc.vector.tensor_add(acc[:, 0:w], acc[:, 0:w], prod[:, 0:w])
                if c in (0, 1, 2):
                    expand_quarter(c + 1)
                if c == len(CJS) - 3:
                    # pre-fold while the tail chunks stream: the remaining
                    # (narrow) chunks only add into acc[0:512]
                    nc.vector.tensor_add(
                        acc[:, 0:512], acc[:, 0:512], acc[:, 512:1024]
                    )

            # final fold: 512 -> F (fp32 at the end), + hop-0 diagonal
            live = 512
            while live > 2 * F:
                h = live // 2
                nc.vector.tensor_add(acc[:, 0:h], acc[:, 0:h], acc[:, h:live])
                live = h
            nc.vector.tensor_add(accf[:, :], acc[:, 0:F], acc[:, F : 2 * F])

            nc.sync.dma_start(out=out[:, :], in_=accf[:, :])

    nc.compile()
    _CACHE["nc"] = nc
    return nc


def _chunk_major(xslab):
    """[3, NH, N, F] bf16 -> flat chunk-major: for each chunk,
    [128 i, 3 k, CJ j, F] contiguous."""
    parts = []
    j0 = 0
    for CJ in CJS:
        blk = xslab[:, :, j0 : j0 + CJ, :]          # [3, NH, CJ, F]
        parts.append(np.ascontiguousarray(blk.transpose(1, 0, 2, 3)).reshape(-1))
        j0 += CJ
    return np.concatenate(parts)


def _make_in_maps(A, X):
    idx = np.arange(NH)
    eye = np.eye(128, dtype=np.float32).astype(BF16_NP)
    Xb = X[:, 1:4].astype(BF16_NP)  # (batch, 3, N, N, F) bf16
    in_maps = []
    for c in range(8):
        b, h = c // 2, c % 2
        lo = h * NH
        xk = _chunk_major(Xb[b, :, lo : lo + NH])
        ab = np.asarray(A[b, lo : lo + NH, :], dtype=np.float32).astype(BF16_NP)
        au = ab.view(np.uint16).astype(np.uint32)
        a2 = ((au << 16) | au).view(np.int32)
        dv = np.ascontiguousarray(X[b, 0, lo + idx, lo + idx, :])
        in_maps.append({"xk": xk, "a2": a2, "d": dv, "eye": eye})
    return in_maps


def run(A, X, trace=False, **kw):
    nc = _build_nc()
    in_maps = _make_in_maps(A, X)
    res = run_bass_kernel_spmd(
        nc, in_maps, core_ids=list(range(8)), trace=trace, **kw
    )
    out = np.empty((BATCH, N, F), dtype=np.float32)
    for c in range(8):
        b, h = c // 2, c % 2
        out[b, h * NH : (h + 1) * NH] = res.results[c]["out"]
    return out, res


def kernel(A, X):
    A = np.asarray(A, dtype=np.float32)
    X = np.asarray(X, dtype=np.float32)
    out, _ = run(A, X, trace=False)
    return out


# revision 63
# speedup vs baseline: 1.1015x; 1.1015x over previous
"""GNN message-passing kernel for Trainium2 (8 NeuronCores).

Reference computation:
    out[b,i,f] = X[b,0,i,i,f] + sum_{k=1..3} sum_j A[b,i,j] * X[b,k,i,j,f]

Sharding: 8 cores = (batch b in 0..3) x (i-half h in 0..1); each core owns
a (b, 128-row i-slab) of the output. Hop 0 only contributes its diagonal,
so only X[b,1:4] (3/4 of X) plus the hop-0 diagonal rows are ever sent to
the device. X is converted to bf16 on the host (rel tol is 2e-2; the
full pipeline measures ~7e-3), halving DMA traffic to ~12.6 MB per core,
and re-laid-out chunk-major so each j-chunk is one fully contiguous DMA.
~12.6 MB / ~360 GB/s HBM-per-core is the ~35 us floor this kernel tracks.

Per-core device pipeline (chunks of CJ j-columns; DMA -> PE -> ACT -> DVE):
  - Every chunk has its OWN SBUF buffer (the whole bf16 X slab fits:
    96 KB/partition), so no DMA trigger ever waits on a buffer and the
    stream runs at pure HBM rate regardless of compute hiccups -- this
    decouples the engines and kills backpressure-induced variance.
  - TensorE: identity-stationary bf16 matmuls (single pass) accumulate
    x1+x2+x3 into PSUM fp32. A garbage-weight warm-up burst (memset tile,
    no DMA dependency) trips the HAM clock gate to 2.4 GHz early.
  - ScalarE (otherwise idle): copies the PSUM fp32 hop sum to SBUF bf16.
  - DVE: expands A[i,j] -> expA[i, j*F+f] during the fill (int32
    pair-packed copy, quartered so nothing blocks), then per chunk ONE
    unit-stride bf16 multiply (2x mode; no broadcast AP / PSUM operand,
    either of which would force 1x) and ONE bf16 add into a running
    1024-wide accumulator (+ a fold for 2048-wide chunks); the
    j-reduction finishes in a short final fold, pre-folded while the
    tail chunks stream.

Measured on 8 axon-tunneled trn2 cores: ~54-56 us HW exec typical
(53.9 us best; run-to-run HBM contention moves the stream between
~32-40 us), rel err 6.4e-3. Baseline this replaced: 112 us (fp32,
DVE-bound). Rejected on measurement: 1024-col matmuls (ISA cap 512),
ldweights=False dedup (walrus ignores it), 64-col or all-zero warm-up
matmuls (don't trip HAM -- too few subarrays / zero-skipped), paired
64j DMA transfers (slower in practice), per-matmul LDWEIGHTS is free
once HAM is warm (reorder window hides it).
"""

import sys

if "/opt/trn_rl_repo" not in sys.path:
    sys.path.insert(0, "/opt/trn_rl_repo")

import ml_dtypes
import numpy as np

import concourse.bacc as bacc
import concourse.bass as bass
import concourse.mybir as mybir
from concourse.bass_utils import run_bass_kernel_spmd
from concourse.tile import TileContext

BATCH, KP1, N, F = 4, 4, 256, 64
NH = N // 2          # 128 rows of output per core (partition dim)
CJS = [8, 8, 16] + [32] * 6 + [16, 8, 8]  # sum = 256; small head + tail
assert sum(CJS) == N
MMCOL = 512          # moving columns per matmul (ISA max)
ACCW = 1024          # running accumulator width (elements per partition)
FP32 = mybir.dt.float32
BF16 = mybir.dt.bfloat16
INT32 = mybir.dt.int32
BF16_NP = ml_dtypes.bfloat16

_CACHE = {}


def _build_nc():
    if "nc" in _CACHE:
        return _CACHE["nc"]
    nc = bacc.Bacc("TRN2", target_bir_lowering=False, debug=False, num_devices=8)
    # chunk-major: all of chunk c (128 i x 3 hops x CJ j x F) contiguous
    xk = nc.dram_tensor("xk", [NH * 3 * N * F], BF16, kind="ExternalInput").ap()
    a2 = nc.dram_tensor("a2", [NH, N], INT32, kind="ExternalInput").ap()
    d = nc.dram_tensor("d", [NH, F], FP32, kind="ExternalInput").ap()
    eye = nc.dram_tensor("eye", [128, 128], BF16, kind="ExternalInput").ap()
    out = nc.dram_tensor("out", [NH, F], FP32, kind="ExternalOutput").ap()

    FH = F // 2  # int32 pairs per j in the expanded-A row
    n_small = sum(1 for cj in CJS if cj < 32)
    n_big = sum(1 for cj in CJS if cj >= 32)

    with TileContext(nc) as tc:
        with (
            tc.tile_pool(name="const", bufs=1) as cpool,
            tc.tile_pool(name="xss", bufs=n_small) as xspool,
            tc.tile_pool(name="xsb", bufs=n_big) as xbpool,
            tc.tile_pool(name="sm", bufs=2) as smpool,
            tc.tile_pool(name="pr", bufs=2) as prpool,
            tc.tile_pool(name="ac", bufs=1) as acpool,
            tc.tile_pool(name="ps", bufs=2, space="PSUM") as pspool,
        ):
            # eye/a2 go FIRST on the sync ring so their packets complete
            # before the big chunk transfers occupy the shared SDMA engines.
            eye_sb = cpool.tile([128, 128], BF16)
            nc.sync.dma_start(out=eye_sb[:, :], in_=eye[:, :])
            a2_sb = cpool.tile([128, N], INT32)
            nc.sync.dma_start(out=a2_sb[:, :], in_=a2[:, :])
            d_sb = cpool.tile([128, F], FP32)
            # d is only needed at the end: ACT ring, off the critical path
            nc.scalar.dma_start(out=d_sb[:, :], in_=d[:, :])

            expa = cpool.tile([128, N * FH], INT32)  # = [128, N*F] bf16
            acc = acpool.tile([128, ACCW], BF16)
            nc.vector.memset(acc[:, :], 0.0)
            accf = acpool.tile([128, F], FP32)

            # A-expansion (DVE): expa[i, j*FH + q] = a2[i, j] (int32 =
            # packed bf16 pair), quartered so the first multiply is not
            # blocked behind one long copy.
            e_step = expa.ap[0][0]
            a_step = a2_sb.ap[0][0]

            def expand_quarter(q):
                NQ = N // 4
                eo = bass.AP(
                    expa.tensor, q * NQ * FH, [[e_step, 128], [FH, NQ], [1, FH]]
                )
                ei = bass.AP(
                    a2_sb.tensor, q * NQ, [[a_step, 128], [1, NQ], [0, FH]]
                )
                nc.vector.tensor_copy(eo, ei)

            expand_quarter(0)

            # PE warm-up on a memset tile: no DMA dependency, so the burst
            # starts at engine boot and HAM reaches 2.4 GHz early. Full
            # 128x128 matmuls -- narrow ones don't trip the detector.
            garbage = cpool.tile([128, 128], BF16)
            nc.gpsimd.memset(garbage[:, :], 0.5)
            warm = pspool.tile([128, max(CJS) * F], FP32, name="ps", tag="ps")
            for _ in range(24):
                nc.tensor.matmul(
                    warm[:, 0:128],
                    garbage[:, :],
                    garbage[:, :],
                    start=True,
                    stop=True,
                )

            # DMA granularity is decoupled from compute granularity: the six
            # 32j body chunks arrive as three 64j transfers (fewer DMA
            # fixed-overhead bubbles in the stream); compute still runs in
            # 32j chunks (PSUM-sized), reading halves of the shared tile.
            xoff = 0
            for c, CJ in enumerate(CJS):
                CF = CJ * F
                pool = xbpool if CJ >= 32 else xspool
                xt = pool.tile([128, 3 * CF], BF16, name="xt", tag="xt")
                src = bass.AP(xk.tensor, xoff, [[3 * CF, 128], [1, 3 * CF]])
                nc.sync.dma_start(out=xt[:, :], in_=src)
                xoff += 128 * 3 * CF
                hs, xbase = CF, 0

                # hop sum on TensorEngine: bf16 identity matmuls, PSUM fp32
                ps = pspool.tile([128, CF], FP32, name="ps", tag="ps")
                wmm = min(MMCOL, CF)
                for s in range(CF // wmm):
                    sl = slice(s * wmm, (s + 1) * wmm)
                    for k in range(3):
                        nc.tensor.matmul(
                            ps[:, sl],
                            eye_sb[:, :],
                            xt[
                                :,
                                xbase + k * hs + s * wmm : xbase
                                + k * hs
                                + (s + 1) * wmm,
                            ],
                            start=(k == 0),
                            stop=(k == 2),
                        )

                # DVE multiply by expA. Mid-stream chunks go through a
                # ScalarE PSUM->SBUF bf16 copy first so the mul runs in 2x
                # mode; the LAST (tiny) chunk skips that stage -- a 1x
                # PSUM-source mul is cheaper than an extra cross-engine
                # latency hop on the drain-critical path.
                j0 = sum(CJS[:c])
                prod = prpool.tile([128, CF], BF16, name="prod", tag="prod")
                ea = expa[:, j0 * FH : (j0 + CJ) * FH].bitcast(BF16)
                if c == len(CJS) - 1:
                    nc.vector.tensor_mul(prod[:, :], ps[:, :], ea)
                else:
                    s_sb = smpool.tile([128, CF], BF16, name="ssb", tag="ssb")
                    nc.scalar.copy(s_sb[:, :], ps[:, :])
                    nc.vector.tensor_mul(prod[:, :], s_sb[:, :], ea)
                if c == 5:
                    # hop-0 diagonal folded in mid-stream (DVE has slack
                    # here) instead of on the drain-critical final chain
                    nc.vector.tensor_add(acc[:, 0:F], acc[:, 0:F], d_sb[:, :])

                # fold prod down to the acc width, then accumulate (a
                # narrower prod adds into a prefix -- column sums survive)
                w = CF
                while w > ACCW:
                    h = w // 2
                    nc.vector.tensor_add(prod[:, 0:h], prod[:, 0:h], prod[:, h:w])
                    w = h
                nc.vector.tensor_add(acc[:, 0:w], acc[:, 0:w], prod[:, 0:w])
                if c in (0, 1, 2):
                    expand_quarter(c + 1)
                if c == len(CJS) - 3:
                    # pre-fold while the tail chunks stream: the remaining
                    # (narrow) chunks only add into acc[0:512]
                    nc.vector.tensor_add(
                        acc[:, 0:512], acc[:, 0:512], acc[:, 512:1024]
                    )

            # final fold: 512 -> F (fp32 at the end), + hop-0 diagonal
            live = 512
            while live > 2 * F:
                h = live // 2
                nc.vector.tensor_add(acc[:, 0:h], acc[:, 0:h], acc[:, h:live])
                live = h
            nc.vector.tensor_add(accf[:, :], acc[:, 0:F], acc[:, F : 2 * F])

            nc.sync.dma_start(out=out[:, :], in_=accf[:, :])

    nc.compile()
    _CACHE["nc"] = nc
    return nc


def _chunk_major(xslab):
    """[3, NH, N, F] bf16 -> flat chunk-major: for each chunk,
    [128 i, 3 k, CJ j, F] contiguous."""
    parts = []
    j0 = 0
    for CJ in CJS:
        blk = xslab[:, :, j0 : j0 + CJ, :]          # [3, NH, CJ, F]
        parts.append(np.ascontiguousarray(blk.transpose(1, 0, 2, 3)).reshape(-1))
        j0 += CJ
    return np.concatenate(parts)


def _make_in_maps(A, X):
    idx = np.arange(NH)
    eye = np.eye(128, dtype=np.float32).astype(BF16_NP)
    Xb = X[:, 1:4].astype(BF16_NP)  # (batch, 3, N, N, F) bf16
    in_maps = []
    for c in range(8):
        b, h = c // 2, c % 2
        lo = h * NH
        xk = _chunk_major(Xb[b, :, lo : lo + NH])
        ab = np.asarray(A[b, lo : lo + NH, :], dtype=np.float32).astype(BF16_NP)
        au = ab.view(np.uint16).astype(np.uint32)
        a2 = ((au << 16) | au).view(np.int32)
        dv = np.ascontiguousarray(X[b, 0, lo + idx, lo + idx, :])
        in_maps.append({"xk": xk, "a2": a2, "d": dv, "eye": eye})
    return in_maps


def run(A, X, trace=False, **kw):
    nc = _build_nc()
    in_maps = _make_in_maps(A, X)
    res = run_bass_kernel_spmd(
        nc, in_maps, core_ids=list(range(8)), trace=trace, **kw
    )
    out = np.empty((BATCH, N, F), dtype=np.float32)
    for c in range(8):
        b, h = c // 2, c % 2
        out[b, h * NH : (h + 1) * NH] = res.results[c]["out"]
    return out, res


def kernel(A, X):
    A = np.asarray(A, dtype=np.float32)
    X = np.asarray(X, dtype=np.float32)
    out, _ = run(A, X, trace=False)
    return out
